# revision 9
# baseline (speedup 1.0000x reference)
"""AdaptiveEMA TRN2 kernel, even/odd-interleaved scan, block layout.

Recurrence split halves the DVE scan length (the kernel's hard bottleneck —
the scan runs at 2 cycles/element regardless of dtype):
    even chain: y[2i] = a^2*y[2i-2] + v[2i],  v[2i] = a*x[2i-1] + x[2i]
    odd  chain: y[2i+1] = a*y[2i] + x[2i+1]   (elementwise)
v is produced by TensorE diagonal matmuls directly into PSUM and the DVE scan
reads PSUM. Truncation correction + normalization (exact identity
y[t] - aK*y[t-K], weights diag(invc)/diag(-aK*invc)) also run on TensorE;
ScalarE drains PSUM.

ALL device-side accesses are contiguous: the host de-interleaves x into
even/odd column blocks and re-interleaves the output (strided fp16 writes on
the engines clobber neighbouring columns - 4-byte write granularity).

Output DRAM layout per row: [ out(t even) 0..2047 | out(t odd) 0..2047 ].
"""

import numpy as np

from contextlib import ExitStack

import concourse.bass as bass
import concourse.mybir as mybir
import concourse.tile as tile
from concourse import bacc
from concourse.bass_utils import run_bass_kernel_spmd

B, F, S = 32, 256, 4096
MAX_SIZE = 200
K = MAX_SIZE + 1
N_CORES = 8
B_LOC = B // N_CORES
C = B_LOC * F
P = 128
NT = C // P
NPAR = F // P
H = S // 2                # 2048 even/odd elements per chain
RAMP_H = MAX_SIZE // 2    # 100
CORR0 = RAMP_H            # first corrected chain index i=100
NCORR = H - CORR0         # 1948
NCH = 4
CWC = NCORR // NCH        # 487
VW = 512                  # matmul moving-dim chunk
HH = H // 2               # 1024, one vps half

F32 = mybir.dt.float32
F16 = mybir.dt.float16
OP_MULT = mybir.AluOpType.mult
OP_ADD = mybir.AluOpType.add


def build_bass():
    nc = bacc.Bacc("TRN2", target_bir_lowering=False, debug=False, num_devices=N_CORES)

    xe = nc.declare_dram_parameter("xe", [C, H], F16, isOutput=False)
    xo = nc.declare_dram_parameter("xo", [C, H], F16, isOutput=False)
    avec = nc.declare_dram_parameter("avec", [P, NPAR], F32, isOutput=False)
    a2vec = nc.declare_dram_parameter("a2vec", [P, NPAR], F32, isOutput=False)
    dam = nc.declare_dram_parameter("dam", [P, NPAR * P], F16, isOutput=False)
    eym = nc.declare_dram_parameter("eym", [P, P], F16, isOutput=False)
    d1m = nc.declare_dram_parameter("d1m", [P, NPAR * P], F16, isOutput=False)
    d2m = nc.declare_dram_parameter("d2m", [P, NPAR * P], F16, isOutput=False)
    invte = nc.declare_dram_parameter("invte", [P, NPAR * RAMP_H], F32, isOutput=False)
    invto = nc.declare_dram_parameter("invto", [P, NPAR * RAMP_H], F32, isOutput=False)
    out = nc.declare_dram_parameter("out", [C, S], F16, isOutput=True)

    with ExitStack() as ctx:
        tc = ctx.enter_context(tile.TileContext(nc))
        cpool = ctx.enter_context(tc.tile_pool(name="const", bufs=1))
        xepool = ctx.enter_context(tc.tile_pool(name="xep", bufs=4))
        xopool = ctx.enter_context(tc.tile_pool(name="xop", bufs=4))
        yepool = ctx.enter_context(tc.tile_pool(name="ye", bufs=4))
        yopool = ctx.enter_context(tc.tile_pool(name="yo", bufs=4))
        opool = ctx.enter_context(tc.tile_pool(name="op", bufs=4))
        vpool = ctx.enter_context(tc.tile_pool(name="vp", bufs=2, space="PSUM"))
        pspool = ctx.enter_context(tc.tile_pool(name="ps", bufs=4, space="PSUM"))

        # first-v dependencies first: Da, I, then the scan's alpha^2
        da_sb = cpool.tile([P, NPAR * P], F16)
        nc.scalar.dma_start(da_sb[:], dam[:])
        ey_sb = cpool.tile([P, P], F16)
        nc.scalar.dma_start(ey_sb[:], eym[:])
        a2_sb = cpool.tile([P, NPAR], F32)
        nc.scalar.dma_start(a2_sb[:], a2vec[:])
        a_sb = cpool.tile([P, NPAR], F32)
        nc.scalar.dma_start(a_sb[:], avec[:])
        d1_sb = cpool.tile([P, NPAR * P], F16)
        nc.scalar.dma_start(d1_sb[:], d1m[:])
        d2_sb = cpool.tile([P, NPAR * P], F16)
        nc.scalar.dma_start(d2_sb[:], d2m[:])
        invte_sb = cpool.tile([P, NPAR * RAMP_H], F32)
        nc.scalar.dma_start(invte_sb[:], invte[:])
        invto_sb = cpool.tile([P, NPAR * RAMP_H], F32)
        nc.scalar.dma_start(invto_sb[:], invto[:])

        for j in range(NT):
            p = j % NPAR
            rows = slice(j * P, (j + 1) * P)
            pp = slice(p * P, (p + 1) * P)

            # x even block; x[2i] at col i. Finer pieces on the first tile so
            # the first v-matmul chain starts as early as possible.
            nin = 4 if j == 0 else 2
            npc = H // nin
            xet = xepool.tile([P, H], F16)
            # x odd block, two leading zero cols; x[2i+1] at col 2+i
            # (col 1 doubles as the zero for x[-1] and y[-1] shifted reads)
            xot = xopool.tile([P, 2 + H], F16)
            nc.gpsimd._memset_packed(xot[:, 0:2], 0)
            for c in range(nin):
                nc.sync.dma_start(
                    xot[:, 2 + c * npc:2 + (c + 1) * npc],
                    xo[rows, c * npc:(c + 1) * npc])
                nc.sync.dma_start(
                    xet[:, c * npc:(c + 1) * npc],
                    xe[rows, c * npc:(c + 1) * npc])

            # v[2i] = a*x[2i-1] + x[2i] -> PSUM halves of 1024, chunks of 512
            ye = yepool.tile([P, H], F16)
            for h in range(2):
                vps = vpool.tile([P, HH], F32, tag="vps")
                for c in range(2):
                    i0 = c * VW
                    g0 = h * HH + i0
                    nc.tensor.matmul(
                        vps[:, i0:i0 + VW], da_sb[:, pp],
                        xot[:, 1 + g0:1 + g0 + VW],
                        start=True, stop=False,
                    )
                    nc.tensor.matmul(
                        vps[:, i0:i0 + VW], ey_sb[:],
                        xet[:, g0:g0 + VW],
                        start=False, stop=True,
                    )
                # even chain: ye[i] = a^2*ye[i-1] + v[2i]
                nc.vector.tensor_tensor_scan(
                    out=ye[:, h * HH:(h + 1) * HH],
                    data0=a2_sb[:, p:p + 1].broadcast_to([P, HH]),
                    data1=vps[:],
                    initial=0.0 if h == 0 else ye[:, HH - 1:HH],
                    op0=OP_MULT,
                    op1=OP_ADD,
                )

            # odd chain: yo[2+i] = y[2i+1] = a*ye[i] + x[2i+1]; yo[:,1] = 0
            # (two packed-mode ops beat one 1x scalar_tensor_tensor)
            yo = yopool.tile([P, 2 + H], F16)
            nc.gpsimd._memset_packed(yo[:, 0:2], 0)
            nc.vector.tensor_scalar_mul(yo[:, 2:2 + H], ye[:], a_sb[:, p:p + 1])
            nc.vector.tensor_add(yo[:, 2:2 + H], yo[:, 2:2 + H], xot[:, 2:2 + H])

            # output tile, block layout: [even 0..2047 | odd 0..2047]
            ot = opool.tile([P, S], F16)
            # ramp t<200: even t=2i i<100; odd t=2i+1 i<100
            nc.vector.tensor_mul(
                ot[:, 0:RAMP_H], ye[:, 0:RAMP_H],
                invte_sb[:, p * RAMP_H:(p + 1) * RAMP_H],
            )
            nc.vector.tensor_mul(
                ot[:, H:H + RAMP_H], yo[:, 2:2 + RAMP_H],
                invto_sb[:, p * RAMP_H:(p + 1) * RAMP_H],
            )
            # steady correction, chain index i in [100, 2048):
            # even t=2i:   invc*ye[i]   + (-aK*invc)*y[2i-201]; y[2i-201]=yo[2+i-101]
            # odd  t=2i+1: invc*yo[2+i] + (-aK*invc)*y[2i-200]; y[2i-200]=ye[i-100]
            for c in range(NCH):
                i0 = CORR0 + c * CWC
                ps = pspool.tile([P, CWC], F32, tag="psc")
                nc.tensor.matmul(
                    ps[:], d1_sb[:, pp], ye[:, i0:i0 + CWC],
                    start=True, stop=False,
                )
                nc.tensor.matmul(
                    ps[:], d2_sb[:, pp], yo[:, i0 - 99:i0 - 99 + CWC],
                    start=False, stop=True,
                )
                nc.scalar.copy(ot[:, i0:i0 + CWC], ps[:])

                ps2 = pspool.tile([P, CWC], F32, tag="psc")
                nc.tensor.matmul(
                    ps2[:], d1_sb[:, pp], yo[:, 2 + i0:2 + i0 + CWC],
                    start=True, stop=False,
                )
                nc.tensor.matmul(
                    ps2[:], d2_sb[:, pp], ye[:, i0 - RAMP_H:i0 - RAMP_H + CWC],
                    start=False, stop=True,
                )
                nc.scalar.copy(ot[:, H + i0:H + i0 + CWC], ps2[:])
            nc.scalar.dma_start(out[rows, :], ot[:])

    nc.finalize()
    return nc


_NC_CACHE = None


def _get_nc():
    global _NC_CACHE
    if _NC_CACHE is None:
        _NC_CACHE = build_bass()
    return _NC_CACHE


def _host_params(log_halflife):
    lh = log_halflife.astype(np.float64)
    alpha = 0.5 ** (1.0 / np.exp(lh))                     # [F]
    aK = alpha ** K
    powers = alpha[:, None] ** np.arange(K, dtype=np.float64)[None, :]
    csum = np.cumsum(powers, axis=1)
    inv_all = 1.0 / (csum + 1e-8)                          # [F, K]
    invc = inv_all[:, MAX_SIZE]

    def fold(v):
        return np.ascontiguousarray(
            v.reshape(NPAR, P, *v.shape[1:]).swapaxes(0, 1)
        )

    avec = fold(alpha).astype(np.float32)
    a2vec = fold(alpha * alpha).astype(np.float32)
    invte = fold(inv_all[:, 0:MAX_SIZE:2]).reshape(P, NPAR * RAMP_H).astype(np.float32)
    invto = fold(inv_all[:, 1:MAX_SIZE:2]).reshape(P, NPAR * RAMP_H).astype(np.float32)
    dam = np.zeros((P, NPAR, P), np.float16)
    d1m = np.zeros((P, NPAR, P), np.float16)
    d2m = np.zeros((P, NPAR, P), np.float16)
    idx = np.arange(P)
    for p in range(NPAR):
        dam[idx, p, idx] = alpha[p * P:(p + 1) * P].astype(np.float16)
        d1m[idx, p, idx] = invc[p * P:(p + 1) * P].astype(np.float16)
        d2m[idx, p, idx] = (-aK * invc)[p * P:(p + 1) * P].astype(np.float16)
    eym = np.eye(P, dtype=np.float16)
    return dict(
        avec=avec, a2vec=a2vec,
        dam=dam.reshape(P, NPAR * P), eym=eym,
        d1m=d1m.reshape(P, NPAR * P), d2m=d2m.reshape(P, NPAR * P),
        invte=invte, invto=invto,
    )


def run(x, log_halflife, trace=False):
    x = np.asarray(x)
    log_halflife = np.asarray(log_halflife, dtype=np.float32)
    assert x.shape == (B, F, S) and log_halflife.shape == (F,)

    params = _host_params(log_halflife)
    x16 = x.astype(np.float16)
    in_maps = []
    for i in range(N_CORES):
        shard = x16[i * B_LOC:(i + 1) * B_LOC].reshape(C, S)
        in_maps.append({
            "xe": np.ascontiguousarray(shard[:, 0::2]),
            "xo": np.ascontiguousarray(shard[:, 1::2]),
            **params,
        })

    nc = _get_nc()
    res = run_bass_kernel_spmd(nc, in_maps, core_ids=list(range(N_CORES)), trace=trace)
    full = np.empty((B, F, S), dtype=np.float32)
    for i in range(N_CORES):
        blk = res.results[i]["out"].astype(np.float32).reshape(B_LOC, F, 2, H)
        dst = full[i * B_LOC:(i + 1) * B_LOC].reshape(B_LOC, F, H, 2)
        dst[:, :, :, 0] = blk[:, :, 0, :]
        dst[:, :, :, 1] = blk[:, :, 1, :]
    return full, res.exec_time_ns


def kernel(x, log_halflife):
    out, _ = run(x, log_halflife, trace=False)
    return out
